# revision 37
# baseline (speedup 1.0000x reference)
"""Bass/Trainium2 kernel for nn_AttentionLayer_68229850464552.

Full multi-head causal attention layer (QKV proj + partial RoPE + attention +
output proj), head-sharded (tensor parallel) across 8 NeuronCores. Each core
computes 2 of the 16 heads for both batch elements and the partial output
projection for its heads' feature columns; the host sums the 8 partials and
adds the output bias.

Matmul operands are bf16 (PE streams 2B/lane/cycle -> 1 cycle/row; fp32/fp32r
stream at half rate); accumulation is fp32 in PSUM throughout.

v3 changes vs v2 (358us -> ~353us):
 - PE warmup runs on memset tiles (no DMA deps): first matmul at ~7.5us
   (runtime-preamble floor) instead of 11us, HAM clock-gate warm before the
   first real work
 - kt-major weight layout + kt-pair DMA splits for x chunks / wq / wv; all
   of batch 0's projection chunks run kt-outer across 4 PSUM accumulators,
   consuming each kt-pair as its DMA lands (DMA-paced, no idle windows)
 - strict two-queue DMA ordering by first use (sync: x stream, scalar:
   weights) so late-use tensors can't steal early bandwidth
 - softmax denominators for full key blocks via fp8-e4m3 DoubleRow matmuls
   (2 key blocks per PE pass; probabilities scaled 1/16 into fp8, ones
   weights = 16 to compensate exactly); adds ~8e-5 rel err
 - out-proj evac engine alternates by column slice (n%2), not (sblk+n)%2,
   so po-slice WAW deps stay same-engine (kills a cross-engine scalar/
   vector convoy that stalled the PE ~3us at the batch boundary)
 - batch 1 chunk 0 QKV also kt-outer on the acc pool, decoupling it from
   the out-proj psC bank rotation (same convoy, second site)
 - attention query-chunks run in order [1,0,2,3]; a finished chunk's
   out-proj is emitted right after each unit's first score matmul, filling
   the exposed first-exp latency at unit start (batch 1's first two units
   use batch 0's deferred final out-proj, split by half)
 - v ones columns dropped (denominator comes from the ones matmul / fp8
   DoubleRow path)

Self-contained: hardcodes shapes from the problem spec.
"""
import os
import numpy as np
import ml_dtypes
from contextlib import ExitStack

import concourse.bass as bass
import concourse.mybir as mybir
import concourse.tile as tile
from concourse import bacc
from concourse.bass_utils import run_bass_kernel_spmd

B, S, D, H, DK = 2, 2048, 2048, 16, 128
HPC = 2                      # heads per core
NCORES = 8
DR = 32                      # rope features
SCALE = 1.0 / float(np.sqrt(DK))
CH = 512                     # x seq-chunk width for the QKV projection
NCH = S // CH                # 4
QCW = 512                    # query chunk width in attention
NQC = S // QCW               # 4
NJ = S // 128                # 16 key blocks
WQ_COLS = 4 * 128            # q0,q1,k0,k1 M-tiles
WV_COLS = 2 * 128            # [v_h0 | v_h1]

F32 = mybir.dt.float32
BF16 = mybir.dt.bfloat16
FP8 = mybir.dt.float8e4
Act = mybir.ActivationFunctionType
Alu = mybir.AluOpType
BF_NP = ml_dtypes.bfloat16

_PROG_CACHE = {}


def _build_program():
    nc = bacc.Bacc("TRN2", target_bir_lowering=False, debug=False,
                   enable_asserts=True, num_devices=NCORES)

    # host-side layouts match the SBUF tile layouts exactly (contiguous DMA)
    xH = nc.dram_tensor("xH", [B, NCH, 128, 16, CH], BF16,
                        kind="ExternalInput").ap()
    # wq is kt-major on host: [kt-pair, part, kt-in-pair, mt, j] so the DMA
    # stream delivers every M-tile's k-slice together (chunk-0 runs kt-outer
    # at DMA pace)
    wq = nc.dram_tensor("wq", [8, 128, 2, 4, 128], BF16,
                        kind="ExternalInput").ap()
    wv = nc.dram_tensor("wv", [8, 128, 2, WV_COLS], BF16,
                        kind="ExternalInput").ap()
    wo = nc.dram_tensor("wo", [128, 2, D], BF16, kind="ExternalInput").ap()
    bqk = nc.dram_tensor("bqk", [128, 4], F32, kind="ExternalInput").ap()
    bv = nc.dram_tensor("bv", [128, WV_COLS], F32, kind="ExternalInput").ap()
    cosT = nc.dram_tensor("cosT", [DR, S], BF16, kind="ExternalInput").ap()
    sinT = nc.dram_tensor("sinT", [DR, S], BF16, kind="ExternalInput").ap()
    maskT = nc.dram_tensor("maskT", [128, 128], BF16, kind="ExternalInput").ap()
    idm = nc.dram_tensor("idm", [128, 128], BF16, kind="ExternalInput").ap()
    pout = nc.dram_tensor("pout", [B * S // 128, 128, D], BF16,
                          kind="ExternalOutput").ap()

    with tile.TileContext(nc) as tc, ExitStack() as ctx:
        wpool = ctx.enter_context(tc.tile_pool(name="w", bufs=1))
        xpool = ctx.enter_context(tc.tile_pool(name="x", bufs=3))
        qkpool = ctx.enter_context(tc.tile_pool(name="qk", bufs=2))
        vpool = ctx.enter_context(tc.tile_pool(name="v", bufs=2))
        otpool = ctx.enter_context(tc.tile_pool(name="ot", bufs=1))
        ppool = ctx.enter_context(tc.tile_pool(name="p", bufs=4))
        rpool = ctx.enter_context(tc.tile_pool(name="r", bufs=3))
        opool = ctx.enter_context(tc.tile_pool(name="o", bufs=4))
        scpool = ctx.enter_context(tc.tile_pool(name="sc", bufs=2, space="PSUM"))
        accpool = ctx.enter_context(tc.tile_pool(name="acc", bufs=4, space="PSUM"))
        pjpool = ctx.enter_context(tc.tile_pool(name="pj", bufs=2, space="PSUM"))

        # ---- PE warmup path: no DMA dependencies at all. A memset ones
        # tile feeds junk matmuls immediately, so the HAM clock-gate warms
        # while the first real operands stream in.
        ones_sb = wpool.tile([128, 256], BF16)
        nc.gpsimd.memset(ones_sb[:], 1.0)
        # DoubleRow stationary for the paired softmax-denominator matmuls:
        # value 16 compensates the 1/16 scale on the fp8 probabilities
        # (both powers of two -> exact)
        ones8_sb = wpool.tile([128, 2, 128], FP8)
        nc.gpsimd.memset(ones8_sb[:], 16.0)
        warm_ps = scpool.tile([128, 512], F32, tag="sc", name="warm")
        for _ in range(10):
            nc.tensor.matmul(warm_ps[:, 0:256], ones_sb[:, 0:128], ones_sb[:],
                             start=True, stop=True)

        # ---- input DMAs. Two HWDGE queues issue in parallel, ordered by
        # first use; everything not needed in the first ~30us is queued
        # BEHIND the chunk-0/1 stream so it cannot steal DMA bandwidth.
        #   sync:   bqk, xt0 (kt-pairs), xt_c1 (kt-pairs), xt_c2, xt_c3
        #   scalar: wq (kt-pairs), wv (kt-pairs), bv, cos, sin, wo
        #   gpsimd: maskT, idm (tiny, needed by first attention only)
        # PE consumes the two streams kt-outer, so chunk 0+1 projections run
        # at DMA pace (~1.7us per kt-pair) with no idle window for the HAM
        # clock-gate to re-throttle.
        bqk_sb = wpool.tile([128, 4], F32)
        nc.sync.dma_start(bqk_sb[:], bqk[:])
        wq_sb = wpool.tile([128, 16, 4, 128], BF16)
        wv_sb = wpool.tile([128, 16, WV_COLS], BF16)
        xt0 = xpool.tile([128, 16, CH], BF16, tag="xt", name="xt0")
        for i in range(8):
            nc.scalar.dma_start(wq_sb[:, 2 * i:2 * i + 2], wq[i])
            nc.sync.dma_start(xt0[:, 2 * i:2 * i + 2],
                              xH[0, 0][:, 2 * i:2 * i + 2])
        # wv pairs split across both queues right behind the main streams
        for i in range(8):
            q = nc.sync if i % 2 == 0 else nc.scalar
            q.dma_start(wv_sb[:, 2 * i:2 * i + 2], wv[i])
        xt_b0 = {0: xt0}
        xt_b0[1] = xpool.tile([128, 16, CH], BF16, tag="xt", name="xt_c1")
        for i in range(8):
            nc.sync.dma_start(xt_b0[1][:, 2 * i:2 * i + 2],
                              xH[0, 1][:, 2 * i:2 * i + 2])
        cos_sb = wpool.tile([DR, S], BF16)
        nc.scalar.dma_start(cos_sb[:], cosT[:])
        sin_sb = wpool.tile([DR, S], BF16)
        nc.scalar.dma_start(sin_sb[:], sinT[:])
        bv_sb = wpool.tile([128, WV_COLS], F32)
        nc.scalar.dma_start(bv_sb[:], bv[:])
        maskT_sb = wpool.tile([128, 128], BF16)
        nc.gpsimd.dma_start(maskT_sb[:], maskT[:])
        idm_sb = wpool.tile([128, 128], BF16)
        nc.gpsimd.dma_start(idm_sb[:], idm[:])
        for c23 in (2, 3):
            xt_b0[c23] = xpool.tile([128, 16, CH], BF16, tag="xt",
                                    name=f"xt_c{c23}")
            for i in range(8):
                nc.sync.dma_start(xt_b0[c23][:, 2 * i:2 * i + 2],
                                  xH[0, c23][:, 2 * i:2 * i + 2])
        # wo is not needed until the first out-projection (~95us in); keep it
        # off the early DMA window entirely
        wo_sb = wpool.tile([128, 2, D], BF16)
        nc.scalar.dma_start(wo_sb[:], wo[:])

        def outproj_fn(b, ot_sb):
            def _outproj(qc=NQC - 1, half=None):
                if half is None:
                    sblks = range(4 * qc, 4 * qc + 4)
                elif half == 0:
                    sblks = range(4 * qc, 4 * qc + 2)
                else:
                    sblks = range(4 * qc + 2, 4 * qc + 4)
                tail = half is None and qc == NQC - 1 and b == B - 1
                for sblk in sblks:
                    po = opool.tile([128, D], BF16, tag="po", name="po")
                    for n in range(D // 512):
                        ps = pjpool.tile([128, 512], F32, tag="pj",
                                         name="psC")
                        for kt in range(2):
                            nc.tensor.matmul(
                                ps[:],
                                ot_sb[:, kt, sblk * 128:(sblk + 1) * 128],
                                wo_sb[:, kt, n * 512:(n + 1) * 512],
                                start=(kt == 0), stop=(kt == 1))
                        if n % 2 == 0:
                            nc.vector.tensor_copy(
                                po[:, n * 512:(n + 1) * 512], ps[:])
                        else:
                            nc.scalar.activation(
                                po[:, n * 512:(n + 1) * 512], ps[:], Act.Copy)
                        last = sblk == 4 * qc + 3
                        if tail and (n >= 1 if last else n % 2 == 1):
                            # drain the very last row-blocks in halves (the
                            # final sblk in quarters) so the last DMA flush
                            # overlaps the remaining evacs
                            lo = n * 512 if (last and n >= 2) else (n - 1) * 512
                            nc.sync.dma_start(
                                pout[b * (S // 128) + sblk][:,
                                                            lo:(n + 1) * 512],
                                po[:, lo:(n + 1) * 512])
                    if not tail:
                        nc.sync.dma_start(pout[b * (S // 128) + sblk], po[:])
            return _outproj

        pending = []
        prefetched = None
        for b in range(B):
            # ---------------- Phase A: QKV projection + RoPE ----------------
            # qk_sb[t]: [feat(128), S] for t in (q_h0, q_h1, k_h0, k_h1)
            qk_sb = [qkpool.tile([128, S], BF16, tag=f"qk{t}", name=f"qk{t}")
                     for t in range(4)]
            v_sb = vpool.tile([128, NJ, WV_COLS], BF16, tag="v")

            for c in range(NCH):
                cs = slice(c * CH, (c + 1) * CH)
                if b == 0:
                    xt = xt_b0[c]
                elif c == 0:
                    xt = prefetched
                else:
                    xt = xpool.tile([128, 16, CH], BF16, tag="xt")
                    nc.sync.dma_start(xt[:], xH[b, c])

                if b == 0 or c == 0:
                    # b0 chunks 0/1 run kt-outer across 4 PSUM accumulators
                    # so each kt-pair is consumed as its DMA lands; b1 chunk 0
                    # uses the same path so its PSUM tiles come from the acc
                    # pool, decoupled from the outproj psC bank rotation (a
                    # pj-bank WAR there chains batch-1's first matmuls into
                    # the evac convoy at the batch boundary)
                    ps4 = [accpool.tile([128, CH], F32, tag="acc",
                                        name=f"c{c}acc{mt}") for mt in range(4)]
                    for kt in range(16):
                        for mt in range(4):
                            nc.tensor.matmul(
                                ps4[mt][:], wq_sb[:, kt, mt, :],
                                xt[:, kt, :], start=(kt == 0), stop=(kt == 15))
                    for mt in range(4):
                        nc.scalar.activation(qk_sb[mt][:, cs], ps4[mt][:],
                                             Act.Identity,
                                             bias=bqk_sb[:, mt:mt + 1])
                else:
                    for mt in range(4):
                        ps = pjpool.tile([128, CH], F32, tag="pj")
                        for kt in range(16):
                            nc.tensor.matmul(
                                ps[:], wq_sb[:, kt, mt, :],
                                xt[:, kt, :], start=(kt == 0), stop=(kt == 15))
                        nc.scalar.activation(qk_sb[mt][:, cs], ps[:],
                                             Act.Identity,
                                             bias=bqk_sb[:, mt:mt + 1])

                # RoPE on the first DR features of each q/k tensor, per chunk:
                # rot = [q[16:32] (sign folded into sinT), q[0:16]]
                for t4 in range(4):
                    shuf = rpool.tile([DR, CH], BF16, tag="shuf", name="shuf")
                    nc.sync.dma_start(shuf[0:16, :], qk_sb[t4][16:32, cs])
                    nc.sync.dma_start(shuf[16:32, :], qk_sb[t4][0:16, cs])
                    tmp = rpool.tile([DR, CH], BF16, tag="rt", name="tmp")
                    nc.vector.tensor_tensor(tmp[:], shuf[:], sin_sb[:, cs],
                                            Alu.mult)
                    tgt = qk_sb[t4][0:DR, cs]
                    nc.vector.tensor_tensor(tgt, tgt, cos_sb[:, cs], Alu.mult)
                    nc.vector.tensor_tensor(tgt, tgt, tmp[:], Alu.add)

                # V projection for this chunk ([seq, feat] layout)
                if b == 0 and c == 0:
                    # kt-outer: consume the wv kt-pair stream as it lands
                    psv4 = [accpool.tile([128, WV_COLS], F32, tag="acc",
                                         name=f"v0acc{s2}") for s2 in range(4)]
                    for kt in range(16):
                        for s2 in range(4):
                            nc.tensor.matmul(
                                psv4[s2][:], xt[:, kt, s2 * 128:(s2 + 1) * 128],
                                wv_sb[:, kt, :], start=(kt == 0),
                                stop=(kt == 15))
                    for s2 in range(4):
                        nc.vector.tensor_tensor(
                            v_sb[:, s2, :], psv4[s2][:], bv_sb[:], Alu.add)
                else:
                    for s2 in range(CH // 128):
                        psv = pjpool.tile([128, WV_COLS], F32, tag="pj")
                        for kt in range(16):
                            nc.tensor.matmul(
                                psv[:], xt[:, kt, s2 * 128:(s2 + 1) * 128],
                                wv_sb[:, kt, :], start=(kt == 0),
                                stop=(kt == 15))
                        nc.vector.tensor_tensor(
                            v_sb[:, c * (CH // 128) + s2, :], psv[:],
                            bv_sb[:], Alu.add)


            # prefetch next batch's first x chunk during attention
            if b + 1 < B:
                xt_next = xpool.tile([128, 16, CH], BF16, tag="xt",
                                     name="xt_next")
                nc.sync.dma_start(xt_next[:], xH[b + 1, 0])
            else:
                xt_next = None

            # ------- Phase B + C: attention, pipelined with out-proj --------
            ot_sb = otpool.tile([128, HPC, S], BF16, tag="ot")
            rsums = otpool.tile([128, NQC * HPC, QCW], F32, tag="rsm")

            def norm_h(qc, h, otps, sums):
                i_qh = qc * HPC + h
                nc.vector.reciprocal_approx_fast(rsums[:, i_qh, :], sums[:])
                nc.vector.tensor_tensor(
                    ot_sb[:, h, qc * QCW:(qc + 1) * QCW],
                    otps[:], rsums[:, i_qh, :], Alu.mult)

            def outproj(qc, half):
                outproj_fn(b, ot_sb)(qc, half)

            # qc order [1,0,2,3]: every unit except the first can overlap a
            # finished chunk's out-projection (qc=0 first would leave two
            # units with nothing to fill the first-exp latency)
            for qc, qc_fill in ((1, None), (0, 1), (2, 0), (3, 2)):
                jmax = 4 * qc + 3
                for h in range(HPC):
                    otps = accpool.tile([128, QCW], F32, tag="acc")
                    sums = accpool.tile([128, QCW], F32, tag="acc")

                    def emit_score(j):
                        c0 = (j - 4 * qc) * 128 if j >= 4 * qc else 0
                        diag = j >= 4 * qc
                        sps = scpool.tile([128, QCW], F32, tag="sc",
                                          name="sps")
                        nc.tensor.matmul(
                            sps[:, c0:QCW], qk_sb[2 + h][:, j * 128:(j + 1) * 128],
                            qk_sb[h][:, qc * QCW + c0:(qc + 1) * QCW],
                            start=True, stop=not diag)
                        if diag:
                            # add -1e4 above the diagonal of the diag subblock
                            nc.tensor.matmul(
                                sps[:, c0:c0 + 128], maskT_sb[:], idm_sb[:],
                                start=False, stop=True)
                        return sps

                    pt8_hold = [None]

                    def emit_consume(j, sps):
                        c0 = (j - 4 * qc) * 128 if j >= 4 * qc else 0
                        # the kernel's very last unit keeps bf16 denominators:
                        # its fp8 copies would land on the vector queue right
                        # where norm(3,1) gates the final out-projection
                        full = j < 4 * qc and not (b == B - 1 and qc == 3
                                                   and h == 1)
                        pt = ppool.tile([128, QCW], BF16, tag="pt", name="pt")
                        nc.scalar.activation(pt[:, c0:QCW], sps[:, c0:QCW],
                                             Act.Exp, scale=SCALE)
                        nc.tensor.matmul(
                            otps[:, c0:QCW],
                            v_sb[:, j, 128 * h:128 * h + 128],
                            pt[:, c0:QCW], start=(j == 0), stop=(j == jmax))
                        if full:
                            # full key blocks: denominator via fp8 DoubleRow
                            # over block pairs (2 key blocks per matmul pass).
                            # pt is scaled by 1/16 into e4m3 (max |logit|*scale
                            # ~4.5 -> pt/16 <= ~6, far from the 240 cap); the
                            # ones weights are 16 to compensate exactly.
                            if j % 2 == 0:
                                pt8_hold[0] = ppool.tile([128, 2, QCW], FP8,
                                                         tag="pt8", name="pt8")
                            nc.vector.tensor_scalar_mul(
                                pt8_hold[0][:, j % 2, :], pt[:], 0.0625)
                            if j % 2 == 1:
                                nc.tensor.matmul(
                                    sums[:], ones8_sb[:], pt8_hold[0][:],
                                    start=(j == 1), stop=False,
                                    perf_mode=mybir.MatmulPerfMode.DoubleRow)
                        else:
                            nc.tensor.matmul(
                                sums[:, c0:QCW], ones_sb[:, 0:128],
                                pt[:, c0:QCW], start=(j == 0), stop=(j == jmax))

                    prev = emit_score(0)
                    # a finished chunk's out-projection fills the PE while
                    # the first exp's latency is exposed at unit start; the
                    # first units of batch 1 use batch 0's deferred final
                    # out-projection (one half per head-unit) instead
                    if qc_fill is not None:
                        outproj(qc_fill, h)
                    elif pending:
                        pending[0](NQC - 1, h)
                        if h == HPC - 1:
                            pending.pop(0)
                    for j in range(1, jmax + 1):
                        cur = emit_score(j)
                        emit_consume(j - 1, prev)
                        prev = cur
                    emit_consume(jmax, prev)
                    norm_h(qc, h, otps, sums)
            pending.append(outproj_fn(b, ot_sb))
            prefetched = xt_next
        while pending:
            pending.pop(0)()

    nc.compile()
    return nc


def kernel(x, W_qkv, b_qkv, W_out, b_out):
    x = np.asarray(x, dtype=np.float32)
    W_qkv = np.asarray(W_qkv, dtype=np.float32)
    b_qkv = np.asarray(b_qkv, dtype=np.float32)
    W_out = np.asarray(W_out, dtype=np.float32)
    b_out = np.asarray(b_out, dtype=np.float32)

    if "prog" not in _PROG_CACHE:
        _PROG_CACHE["prog"] = _build_program()
    nc = _PROG_CACHE["prog"]

    xT = x.transpose(0, 2, 1)                       # [B, D, S]
    xH = np.ascontiguousarray(
        xT.reshape(B, 16, 128, NCH, CH).transpose(0, 3, 2, 1, 4)
    ).astype(BF_NP)                                 # [B, NCH, 128, 16, CH]

    i = np.arange(16, dtype=np.float64)
    theta = 1.0 / (10000.0 ** ((2.0 * i) / DR))
    s_idx = np.arange(S, dtype=np.float64)
    idx = s_idx[:, None] * theta[None, :]          # [S, 16]
    idx2 = np.concatenate([idx, idx], axis=1)      # [S, 32]
    cosT = np.ascontiguousarray(np.cos(idx2).T.astype(np.float32))
    sinT = np.sin(idx2).T.astype(np.float32)
    sinT[0:16, :] *= -1.0          # sign of rot = [-q[16:32], q[0:16]] folded in
    sinT = np.ascontiguousarray(sinT)
    cosT = cosT.astype(BF_NP)
    sinT = sinT.astype(BF_NP)

    maskT = np.triu(np.full((128, 128), -10000.0, dtype=np.float32), 1).astype(BF_NP)
    idm = np.eye(128, dtype=np.float32).astype(BF_NP)

    in_maps = []
    for c in range(NCORES):
        heads = [HPC * c, HPC * c + 1]
        qw, kw, vw, qb, kb, vb = [], [], [], [], [], []
        for hh in heads:
            base = 3 * DK * hh
            qw.append(W_qkv[base:base + 128])
            kw.append(W_qkv[base + 128:base + 256])
            vw.append(W_qkv[base + 256:base + 384])
            qb.append(b_qkv[base:base + 128])
            kb.append(b_qkv[base + 128:base + 256])
            vb.append(b_qkv[base + 256:base + 384])

        M = np.concatenate([qw[0], qw[1], kw[0], kw[1]], axis=0)  # [512, D]
        # kt-major: wq[pair, p, ki, mt, j] = M[mt*128+j, (2*pair+ki)*128+p]
        wq_np = np.ascontiguousarray(
            M.reshape(4, 128, 16, 128).transpose(2, 3, 0, 1)   # [kt,p,mt,j]
            .reshape(8, 2, 128, 4, 128).transpose(0, 2, 1, 3, 4)).astype(BF_NP)

        Mv = np.concatenate([vw[0], vw[1]], axis=0)            # [256, D]
        # kt-major pairs: wv[pair, p, ki, m] = Mv[m, (2*pair+ki)*128+p]
        wv_np = np.ascontiguousarray(
            Mv.T.reshape(8, 2, 128, WV_COLS)
            .transpose(0, 2, 1, 3)).astype(BF_NP)

        bv_np = np.zeros((1, WV_COLS), dtype=np.float32)
        bv_np[0, 0:128] = vb[0]
        bv_np[0, 128:256] = vb[1]
        bv_np = np.ascontiguousarray(np.repeat(bv_np, 128, axis=0))

        bqk_np = np.zeros((128, 4), dtype=np.float32)
        bqk_np[:, 0] = qb[0]
        bqk_np[:, 1] = qb[1]
        bqk_np[:, 2] = kb[0]
        bqk_np[:, 3] = kb[1]

        Mo = np.ascontiguousarray(
            W_out[:, HPC * DK * c: HPC * DK * (c + 1)].T)       # [256, D]
        wo_np = np.ascontiguousarray(
            Mo.reshape(2, 128, D).transpose(1, 0, 2)).astype(BF_NP)

        in_maps.append({
            "xH": xH, "wq": wq_np, "wv": wv_np, "wo": wo_np,
            "bqk": bqk_np, "bv": bv_np, "cosT": cosT, "sinT": sinT,
            "maskT": maskT, "idm": idm,
        })

    trace = os.environ.get("KERNEL_TRACE", "0") == "1"
    res = run_bass_kernel_spmd(nc, in_maps, core_ids=list(range(NCORES)),
                               trace=trace)
    if res.exec_time_ns is not None:
        print(f"HW exec time: {res.exec_time_ns} ns")
        if res.instructions_and_trace is not None:
            print(f"trace: {res.instructions_and_trace[1]}")

    acc = np.zeros((B * S, D), dtype=np.float64)
    for c in range(NCORES):
        acc += res.results[c]["pout"].reshape(B * S, D).astype(np.float64)
    out = (acc + b_out.astype(np.float64)[None, :]).astype(np.float32)
    return out.reshape(B, S, D)



# revision 38
# speedup vs baseline: 1.1896x; 1.1896x over previous
"""Bass/Trainium2 kernel for nn_AttentionLayer_68229850464552.

Full multi-head causal attention layer (QKV proj + partial RoPE + attention +
output proj), head-sharded (tensor parallel) across 8 NeuronCores. Each core
computes 2 of the 16 heads for both batch elements and the partial output
projection for its heads' feature columns; the host sums the 8 partials and
adds the output bias.

Matmul operands are bf16 (PE streams 2B/lane/cycle -> 1 cycle/row; fp32/fp32r
stream at half rate); accumulation is fp32 in PSUM throughout.

v3 changes vs v2 (358us -> ~353us):
 - PE warmup runs on memset tiles (no DMA deps): first matmul at ~7.5us
   (runtime-preamble floor) instead of 11us, HAM clock-gate warm before the
   first real work
 - kt-major weight layout + kt-pair DMA splits for x chunks / wq / wv; all
   of batch 0's projection chunks run kt-outer across 4 PSUM accumulators,
   consuming each kt-pair as its DMA lands (DMA-paced, no idle windows)
 - strict two-queue DMA ordering by first use (sync: x stream, scalar:
   weights) so late-use tensors can't steal early bandwidth
 - softmax denominators for full key blocks via fp8-e4m3 DoubleRow matmuls
   (2 key blocks per PE pass; probabilities scaled 1/16 into fp8, ones
   weights = 16 to compensate exactly); adds ~8e-5 rel err
 - out-proj evac engine alternates by column slice (n%2), not (sblk+n)%2,
   so po-slice WAW deps stay same-engine (kills a cross-engine scalar/
   vector convoy that stalled the PE ~3us at the batch boundary)
 - batch 1 chunk 0 QKV also kt-outer on the acc pool, decoupling it from
   the out-proj psC bank rotation (same convoy, second site)
 - attention query-chunks run in order [1,0,2,3]; a finished chunk's
   out-proj is emitted right after each unit's first score matmul, filling
   the exposed first-exp latency at unit start (batch 1's first two units
   use batch 0's deferred final out-proj, split by half)
 - v ones columns dropped (denominator comes from the ones matmul / fp8
   DoubleRow path)

Self-contained: hardcodes shapes from the problem spec.
"""
import os
import numpy as np
import ml_dtypes
from contextlib import ExitStack

import concourse.bass as bass
import concourse.mybir as mybir
import concourse.tile as tile
from concourse import bacc
from concourse.bass_utils import run_bass_kernel_spmd

B, S, D, H, DK = 2, 2048, 2048, 16, 128
HPC = 2                      # heads per core
NCORES = 8
DR = 32                      # rope features
SCALE = 1.0 / float(np.sqrt(DK))
CH = 512                     # x seq-chunk width for the QKV projection
NCH = S // CH                # 4
QCW = 512                    # query chunk width in attention
NQC = S // QCW               # 4
NJ = S // 128                # 16 key blocks
WQ_COLS = 4 * 128            # q0,q1,k0,k1 M-tiles
WV_COLS = 2 * 128            # [v_h0 | v_h1]

F32 = mybir.dt.float32
BF16 = mybir.dt.bfloat16
FP8 = mybir.dt.float8e4
Act = mybir.ActivationFunctionType
Alu = mybir.AluOpType
BF_NP = ml_dtypes.bfloat16

_PROG_CACHE = {}


def _build_program():
    nc = bacc.Bacc("TRN2", target_bir_lowering=False, debug=False,
                   enable_asserts=True, num_devices=NCORES)

    # host-side layouts match the SBUF tile layouts exactly (contiguous DMA)
    xH = nc.dram_tensor("xH", [B, NCH, 128, 16, CH], BF16,
                        kind="ExternalInput").ap()
    # wq is kt-major on host: [kt-pair, part, kt-in-pair, mt, j] so the DMA
    # stream delivers every M-tile's k-slice together (chunk-0 runs kt-outer
    # at DMA pace)
    wq = nc.dram_tensor("wq", [8, 128, 2, 4, 128], BF16,
                        kind="ExternalInput").ap()
    wv = nc.dram_tensor("wv", [8, 128, 2, WV_COLS], BF16,
                        kind="ExternalInput").ap()
    wo = nc.dram_tensor("wo", [128, 2, D], BF16, kind="ExternalInput").ap()
    bqk = nc.dram_tensor("bqk", [128, 4], F32, kind="ExternalInput").ap()
    bv = nc.dram_tensor("bv", [128, WV_COLS], F32, kind="ExternalInput").ap()
    cosT = nc.dram_tensor("cosT", [DR, S], BF16, kind="ExternalInput").ap()
    sinT = nc.dram_tensor("sinT", [DR, S], BF16, kind="ExternalInput").ap()
    maskT = nc.dram_tensor("maskT", [128, 128], BF16, kind="ExternalInput").ap()
    idm = nc.dram_tensor("idm", [128, 128], BF16, kind="ExternalInput").ap()
    pout = nc.dram_tensor("pout", [B * S // 128, 128, D], BF16,
                          kind="ExternalOutput").ap()

    with tile.TileContext(nc) as tc, ExitStack() as ctx:
        wpool = ctx.enter_context(tc.tile_pool(name="w", bufs=1))
        xpool = ctx.enter_context(tc.tile_pool(name="x", bufs=3))
        qkpool = ctx.enter_context(tc.tile_pool(name="qk", bufs=2))
        vpool = ctx.enter_context(tc.tile_pool(name="v", bufs=2))
        otpool = ctx.enter_context(tc.tile_pool(name="ot", bufs=1))
        ppool = ctx.enter_context(tc.tile_pool(name="p", bufs=4))
        rpool = ctx.enter_context(tc.tile_pool(name="r", bufs=3))
        opool = ctx.enter_context(tc.tile_pool(name="o", bufs=4))
        scpool = ctx.enter_context(tc.tile_pool(name="sc", bufs=2, space="PSUM"))
        accpool = ctx.enter_context(tc.tile_pool(name="acc", bufs=4, space="PSUM"))
        pjpool = ctx.enter_context(tc.tile_pool(name="pj", bufs=2, space="PSUM"))

        # ---- PE warmup path: no DMA dependencies at all. A memset ones
        # tile feeds junk matmuls immediately, so the HAM clock-gate warms
        # while the first real operands stream in.
        ones_sb = wpool.tile([128, 256], BF16)
        nc.gpsimd.memset(ones_sb[:], 1.0)
        # DoubleRow stationary for the paired softmax-denominator matmuls:
        # value 16 compensates the 1/16 scale on the fp8 probabilities
        # (both powers of two -> exact)
        ones8_sb = wpool.tile([128, 2, 128], FP8)
        nc.gpsimd.memset(ones8_sb[:], 16.0)
        warm_ps = scpool.tile([128, 512], F32, tag="sc", name="warm")
        for _ in range(10):
            nc.tensor.matmul(warm_ps[:, 0:256], ones_sb[:, 0:128], ones_sb[:],
                             start=True, stop=True)

        # ---- input DMAs. Two HWDGE queues issue in parallel, ordered by
        # first use; everything not needed in the first ~30us is queued
        # BEHIND the chunk-0/1 stream so it cannot steal DMA bandwidth.
        #   sync:   bqk, xt0 (kt-pairs), xt_c1 (kt-pairs), xt_c2, xt_c3
        #   scalar: wq (kt-pairs), wv (kt-pairs), bv, cos, sin, wo
        #   gpsimd: maskT, idm (tiny, needed by first attention only)
        # PE consumes the two streams kt-outer, so chunk 0+1 projections run
        # at DMA pace (~1.7us per kt-pair) with no idle window for the HAM
        # clock-gate to re-throttle.
        bqk_sb = wpool.tile([128, 4], F32)
        nc.sync.dma_start(bqk_sb[:], bqk[:])
        wq_sb = wpool.tile([128, 16, 4, 128], BF16)
        wv_sb = wpool.tile([128, 16, WV_COLS], BF16)
        xt0 = xpool.tile([128, 16, CH], BF16, tag="xt", name="xt0")
        for i in range(8):
            nc.scalar.dma_start(wq_sb[:, 2 * i:2 * i + 2], wq[i])
            nc.sync.dma_start(xt0[:, 2 * i:2 * i + 2],
                              xH[0, 0][:, 2 * i:2 * i + 2])
        # wv pairs split across both queues right behind the main streams
        for i in range(8):
            q = nc.sync if i % 2 == 0 else nc.scalar
            q.dma_start(wv_sb[:, 2 * i:2 * i + 2], wv[i])
        xt_b0 = {0: xt0}
        xt_b0[1] = xpool.tile([128, 16, CH], BF16, tag="xt", name="xt_c1")
        for i in range(8):
            nc.sync.dma_start(xt_b0[1][:, 2 * i:2 * i + 2],
                              xH[0, 1][:, 2 * i:2 * i + 2])
        cos_sb = wpool.tile([DR, S], BF16)
        nc.scalar.dma_start(cos_sb[:], cosT[:])
        sin_sb = wpool.tile([DR, S], BF16)
        nc.scalar.dma_start(sin_sb[:], sinT[:])
        bv_sb = wpool.tile([128, WV_COLS], F32)
        nc.scalar.dma_start(bv_sb[:], bv[:])
        maskT_sb = wpool.tile([128, 128], BF16)
        nc.gpsimd.dma_start(maskT_sb[:], maskT[:])
        idm_sb = wpool.tile([128, 128], BF16)
        nc.gpsimd.dma_start(idm_sb[:], idm[:])
        for c23 in (2, 3):
            xt_b0[c23] = xpool.tile([128, 16, CH], BF16, tag="xt",
                                    name=f"xt_c{c23}")
            for i in range(8):
                nc.sync.dma_start(xt_b0[c23][:, 2 * i:2 * i + 2],
                                  xH[0, c23][:, 2 * i:2 * i + 2])
        # wo is not needed until the first out-projection (~95us in); keep it
        # off the early DMA window entirely
        wo_sb = wpool.tile([128, 2, D], BF16)
        nc.scalar.dma_start(wo_sb[:], wo[:])

        def outproj_fn(b, ot_sb):
            def _outproj(qc=NQC - 1, half=None):
                if half is None:
                    sblks = range(4 * qc, 4 * qc + 4)
                elif half == 0:
                    sblks = range(4 * qc, 4 * qc + 2)
                else:
                    sblks = range(4 * qc + 2, 4 * qc + 4)
                tail = half is None and qc == NQC - 1 and b == B - 1
                for sblk in sblks:
                    po = opool.tile([128, D], BF16, tag="po", name="po")
                    for n in range(D // 512):
                        ps = pjpool.tile([128, 512], F32, tag="pj",
                                         name="psC")
                        for kt in range(2):
                            nc.tensor.matmul(
                                ps[:],
                                ot_sb[:, kt, sblk * 128:(sblk + 1) * 128],
                                wo_sb[:, kt, n * 512:(n + 1) * 512],
                                start=(kt == 0), stop=(kt == 1))
                        if n % 2 == 0:
                            nc.vector.tensor_copy(
                                po[:, n * 512:(n + 1) * 512], ps[:])
                        else:
                            nc.scalar.activation(
                                po[:, n * 512:(n + 1) * 512], ps[:], Act.Copy)
                        last = sblk == 4 * qc + 3
                        if tail and (n >= 1 if last else n % 2 == 1):
                            # drain the very last row-blocks in halves (the
                            # final sblk in quarters) so the last DMA flush
                            # overlaps the remaining evacs
                            lo = n * 512 if (last and n >= 2) else (n - 1) * 512
                            nc.sync.dma_start(
                                pout[b * (S // 128) + sblk][:,
                                                            lo:(n + 1) * 512],
                                po[:, lo:(n + 1) * 512])
                    if not tail:
                        nc.sync.dma_start(pout[b * (S // 128) + sblk], po[:])
            return _outproj

        pending = []
        prefetched = None
        for b in range(B):
            # ---------------- Phase A: QKV projection + RoPE ----------------
            # qk_sb[t]: [feat(128), S] for t in (q_h0, q_h1, k_h0, k_h1)
            qk_sb = [qkpool.tile([128, S], BF16, tag=f"qk{t}", name=f"qk{t}")
                     for t in range(4)]
            v_sb = vpool.tile([128, NJ, WV_COLS], BF16, tag="v")

            for c in range(NCH):
                cs = slice(c * CH, (c + 1) * CH)
                if b == 0:
                    xt = xt_b0[c]
                elif c == 0:
                    xt = prefetched
                else:
                    xt = xpool.tile([128, 16, CH], BF16, tag="xt")
                    nc.sync.dma_start(xt[:], xH[b, c])

                if b == 0 or c == 0:
                    # b0 chunks 0/1 run kt-outer across 4 PSUM accumulators
                    # so each kt-pair is consumed as its DMA lands; b1 chunk 0
                    # uses the same path so its PSUM tiles come from the acc
                    # pool, decoupled from the outproj psC bank rotation (a
                    # pj-bank WAR there chains batch-1's first matmuls into
                    # the evac convoy at the batch boundary)
                    ps4 = [accpool.tile([128, CH], F32, tag="acc",
                                        name=f"c{c}acc{mt}") for mt in range(4)]
                    for kt in range(16):
                        for mt in range(4):
                            nc.tensor.matmul(
                                ps4[mt][:], wq_sb[:, kt, mt, :],
                                xt[:, kt, :], start=(kt == 0), stop=(kt == 15))
                    for mt in range(4):
                        nc.scalar.activation(qk_sb[mt][:, cs], ps4[mt][:],
                                             Act.Identity,
                                             bias=bqk_sb[:, mt:mt + 1])
                else:
                    for mt in range(4):
                        ps = pjpool.tile([128, CH], F32, tag="pj")
                        for kt in range(16):
                            nc.tensor.matmul(
                                ps[:], wq_sb[:, kt, mt, :],
                                xt[:, kt, :], start=(kt == 0), stop=(kt == 15))
                        nc.scalar.activation(qk_sb[mt][:, cs], ps[:],
                                             Act.Identity,
                                             bias=bqk_sb[:, mt:mt + 1])

                # RoPE on the first DR features of each q/k tensor, per chunk:
                # rot = [q[16:32] (sign folded into sinT), q[0:16]]
                for t4 in range(4):
                    shuf = rpool.tile([DR, CH], BF16, tag="shuf", name="shuf")
                    nc.sync.dma_start(shuf[0:16, :], qk_sb[t4][16:32, cs])
                    nc.sync.dma_start(shuf[16:32, :], qk_sb[t4][0:16, cs])
                    tmp = rpool.tile([DR, CH], BF16, tag="rt", name="tmp")
                    nc.vector.tensor_tensor(tmp[:], shuf[:], sin_sb[:, cs],
                                            Alu.mult)
                    tgt = qk_sb[t4][0:DR, cs]
                    nc.vector.tensor_tensor(tgt, tgt, cos_sb[:, cs], Alu.mult)
                    nc.vector.tensor_tensor(tgt, tgt, tmp[:], Alu.add)

                # V projection for this chunk ([seq, feat] layout)
                if b == 0 and c == 0:
                    # kt-outer: consume the wv kt-pair stream as it lands
                    psv4 = [accpool.tile([128, WV_COLS], F32, tag="acc",
                                         name=f"v0acc{s2}") for s2 in range(4)]
                    for kt in range(16):
                        for s2 in range(4):
                            nc.tensor.matmul(
                                psv4[s2][:], xt[:, kt, s2 * 128:(s2 + 1) * 128],
                                wv_sb[:, kt, :], start=(kt == 0),
                                stop=(kt == 15))
                    for s2 in range(4):
                        nc.vector.tensor_tensor(
                            v_sb[:, s2, :], psv4[s2][:], bv_sb[:], Alu.add)
                else:
                    for s2 in range(CH // 128):
                        psv = pjpool.tile([128, WV_COLS], F32, tag="pj")
                        for kt in range(16):
                            nc.tensor.matmul(
                                psv[:], xt[:, kt, s2 * 128:(s2 + 1) * 128],
                                wv_sb[:, kt, :], start=(kt == 0),
                                stop=(kt == 15))
                        nc.vector.tensor_tensor(
                            v_sb[:, c * (CH // 128) + s2, :], psv[:],
                            bv_sb[:], Alu.add)


            # prefetch next batch's first x chunk during attention
            if b + 1 < B:
                xt_next = xpool.tile([128, 16, CH], BF16, tag="xt",
                                     name="xt_next")
                nc.sync.dma_start(xt_next[:], xH[b + 1, 0])
            else:
                xt_next = None

            # ------- Phase B + C: attention, pipelined with out-proj --------
            ot_sb = otpool.tile([128, HPC, S], BF16, tag="ot")
            rsums = otpool.tile([128, NQC * HPC, QCW], F32, tag="rsm")

            def norm_h(qc, h, otps, sums):
                i_qh = qc * HPC + h
                nc.vector.reciprocal_approx_fast(rsums[:, i_qh, :], sums[:])
                nc.vector.tensor_tensor(
                    ot_sb[:, h, qc * QCW:(qc + 1) * QCW],
                    otps[:], rsums[:, i_qh, :], Alu.mult)

            def outproj(qc, half):
                outproj_fn(b, ot_sb)(qc, half)

            # qc order [1,0,2,3]: every unit except the first can overlap a
            # finished chunk's out-projection (qc=0 first would leave two
            # units with nothing to fill the first-exp latency)
            for qc, qc_fill in ((1, None), (0, 1), (2, 0), (3, 2)):
                jmax = 4 * qc + 3
                for h in range(HPC):
                    otps = accpool.tile([128, QCW], F32, tag="acc")
                    sums = accpool.tile([128, QCW], F32, tag="acc")

                    def emit_score(j):
                        c0 = (j - 4 * qc) * 128 if j >= 4 * qc else 0
                        diag = j >= 4 * qc
                        sps = scpool.tile([128, QCW], F32, tag="sc",
                                          name="sps")
                        nc.tensor.matmul(
                            sps[:, c0:QCW], qk_sb[2 + h][:, j * 128:(j + 1) * 128],
                            qk_sb[h][:, qc * QCW + c0:(qc + 1) * QCW],
                            start=True, stop=not diag)
                        if diag:
                            # add -1e4 above the diagonal of the diag subblock
                            nc.tensor.matmul(
                                sps[:, c0:c0 + 128], maskT_sb[:], idm_sb[:],
                                start=False, stop=True)
                        return sps

                    pt8_hold = [None]

                    def emit_consume(j, sps):
                        c0 = (j - 4 * qc) * 128 if j >= 4 * qc else 0
                        full = j < 4 * qc
                        pt = ppool.tile([128, QCW], BF16, tag="pt", name="pt")
                        nc.scalar.activation(pt[:, c0:QCW], sps[:, c0:QCW],
                                             Act.Exp, scale=SCALE)
                        nc.tensor.matmul(
                            otps[:, c0:QCW],
                            v_sb[:, j, 128 * h:128 * h + 128],
                            pt[:, c0:QCW], start=(j == 0), stop=(j == jmax))
                        if full:
                            # full key blocks: denominator via fp8 DoubleRow
                            # over block pairs (2 key blocks per matmul pass).
                            # pt is scaled by 1/16 into e4m3 (max |logit|*scale
                            # ~4.5 -> pt/16 <= ~6, far from the 240 cap); the
                            # ones weights are 16 to compensate exactly.
                            if j % 2 == 0:
                                pt8_hold[0] = ppool.tile([128, 2, QCW], FP8,
                                                         tag="pt8", name="pt8")
                            nc.vector.tensor_scalar_mul(
                                pt8_hold[0][:, j % 2, :], pt[:], 0.0625)
                            if j % 2 == 1:
                                nc.tensor.matmul(
                                    sums[:], ones8_sb[:], pt8_hold[0][:],
                                    start=(j == 1), stop=False,
                                    perf_mode=mybir.MatmulPerfMode.DoubleRow)
                        else:
                            nc.tensor.matmul(
                                sums[:, c0:QCW], ones_sb[:, 0:128],
                                pt[:, c0:QCW], start=(j == 0), stop=(j == jmax))

                    prev = emit_score(0)
                    # a finished chunk's out-projection fills the PE while
                    # the first exp's latency is exposed at unit start; the
                    # first units of batch 1 use batch 0's deferred final
                    # out-projection (one half per head-unit) instead
                    if qc_fill is not None:
                        outproj(qc_fill, h)
                    elif pending:
                        pending[0](NQC - 1, h)
                        if h == HPC - 1:
                            pending.pop(0)
                    for j in range(1, jmax + 1):
                        cur = emit_score(j)
                        emit_consume(j - 1, prev)
                        prev = cur
                    emit_consume(jmax, prev)
                    norm_h(qc, h, otps, sums)
            pending.append(outproj_fn(b, ot_sb))
            prefetched = xt_next
        while pending:
            pending.pop(0)()

    nc.compile()
    return nc


def kernel(x, W_qkv, b_qkv, W_out, b_out):
    x = np.asarray(x, dtype=np.float32)
    W_qkv = np.asarray(W_qkv, dtype=np.float32)
    b_qkv = np.asarray(b_qkv, dtype=np.float32)
    W_out = np.asarray(W_out, dtype=np.float32)
    b_out = np.asarray(b_out, dtype=np.float32)

    if "prog" not in _PROG_CACHE:
        _PROG_CACHE["prog"] = _build_program()
    nc = _PROG_CACHE["prog"]

    xT = x.transpose(0, 2, 1)                       # [B, D, S]
    xH = np.ascontiguousarray(
        xT.reshape(B, 16, 128, NCH, CH).transpose(0, 3, 2, 1, 4)
    ).astype(BF_NP)                                 # [B, NCH, 128, 16, CH]

    i = np.arange(16, dtype=np.float64)
    theta = 1.0 / (10000.0 ** ((2.0 * i) / DR))
    s_idx = np.arange(S, dtype=np.float64)
    idx = s_idx[:, None] * theta[None, :]          # [S, 16]
    idx2 = np.concatenate([idx, idx], axis=1)      # [S, 32]
    cosT = np.ascontiguousarray(np.cos(idx2).T.astype(np.float32))
    sinT = np.sin(idx2).T.astype(np.float32)
    sinT[0:16, :] *= -1.0          # sign of rot = [-q[16:32], q[0:16]] folded in
    sinT = np.ascontiguousarray(sinT)
    cosT = cosT.astype(BF_NP)
    sinT = sinT.astype(BF_NP)

    maskT = np.triu(np.full((128, 128), -10000.0, dtype=np.float32), 1).astype(BF_NP)
    idm = np.eye(128, dtype=np.float32).astype(BF_NP)

    in_maps = []
    for c in range(NCORES):
        heads = [HPC * c, HPC * c + 1]
        qw, kw, vw, qb, kb, vb = [], [], [], [], [], []
        for hh in heads:
            base = 3 * DK * hh
            qw.append(W_qkv[base:base + 128])
            kw.append(W_qkv[base + 128:base + 256])
            vw.append(W_qkv[base + 256:base + 384])
            qb.append(b_qkv[base:base + 128])
            kb.append(b_qkv[base + 128:base + 256])
            vb.append(b_qkv[base + 256:base + 384])

        M = np.concatenate([qw[0], qw[1], kw[0], kw[1]], axis=0)  # [512, D]
        # kt-major: wq[pair, p, ki, mt, j] = M[mt*128+j, (2*pair+ki)*128+p]
        wq_np = np.ascontiguousarray(
            M.reshape(4, 128, 16, 128).transpose(2, 3, 0, 1)   # [kt,p,mt,j]
            .reshape(8, 2, 128, 4, 128).transpose(0, 2, 1, 3, 4)).astype(BF_NP)

        Mv = np.concatenate([vw[0], vw[1]], axis=0)            # [256, D]
        # kt-major pairs: wv[pair, p, ki, m] = Mv[m, (2*pair+ki)*128+p]
        wv_np = np.ascontiguousarray(
            Mv.T.reshape(8, 2, 128, WV_COLS)
            .transpose(0, 2, 1, 3)).astype(BF_NP)

        bv_np = np.zeros((1, WV_COLS), dtype=np.float32)
        bv_np[0, 0:128] = vb[0]
        bv_np[0, 128:256] = vb[1]
        bv_np = np.ascontiguousarray(np.repeat(bv_np, 128, axis=0))

        bqk_np = np.zeros((128, 4), dtype=np.float32)
        bqk_np[:, 0] = qb[0]
        bqk_np[:, 1] = qb[1]
        bqk_np[:, 2] = kb[0]
        bqk_np[:, 3] = kb[1]

        Mo = np.ascontiguousarray(
            W_out[:, HPC * DK * c: HPC * DK * (c + 1)].T)       # [256, D]
        wo_np = np.ascontiguousarray(
            Mo.reshape(2, 128, D).transpose(1, 0, 2)).astype(BF_NP)

        in_maps.append({
            "xH": xH, "wq": wq_np, "wv": wv_np, "wo": wo_np,
            "bqk": bqk_np, "bv": bv_np, "cosT": cosT, "sinT": sinT,
            "maskT": maskT, "idm": idm,
        })

    trace = os.environ.get("KERNEL_TRACE", "0") == "1"
    res = run_bass_kernel_spmd(nc, in_maps, core_ids=list(range(NCORES)),
                               trace=trace)
    if res.exec_time_ns is not None:
        print(f"HW exec time: {res.exec_time_ns} ns")
        if res.instructions_and_trace is not None:
            print(f"trace: {res.instructions_and_trace[1]}")

    acc = np.zeros((B * S, D), dtype=np.float64)
    for c in range(NCORES):
        acc += res.results[c]["pout"].reshape(B * S, D).astype(np.float64)
    out = (acc + b_out.astype(np.float64)[None, :]).astype(np.float32)
    return out.reshape(B, S, D)

